# revision 3
# baseline (speedup 1.0000x reference)
"""Trainium2 Bass kernel for nn_ColorHistograms.

Pipeline (per core, pure data-parallel over batch):
  frames[int32] -> per-frame 512-bin color histograms (DVE one-hot + PE
  bilinear reduction) -> L2 normalize -> banded self-similarity (PE) ->
  banded gather (strided DMA) -> FC + ReLU -> out.

Histogram method: bin = (r>>5)<<6 | (g>>5)<<3 | (b>>5) = (r3, gb6).
For each pixel build two small one-hots on DVE (8-wide over r3, 64-wide
over gb6, bf16, is_equal vs replicated-iota tables, 2x perf mode), then
one PE matmul per 128-pixel chunk computes
hist[r, gb] += OHr[pix, r]^T @ OHgb[pix, gb] with fp32 PSUM accumulation
(exact integer counts).
"""

import numpy as np

import concourse.bacc as bacc
import concourse.bass as bass
import concourse.mybir as mybir
from concourse import bass_utils
from concourse.tile import TileContext

dt = mybir.dt
Alu = mybir.AluOpType
Act = mybir.ActivationFunctionType

# Problem constants (hardcoded per spec).
B_FULL, T_FULL = 16, 512
H, W_IMG, C = 27, 48, 3
NPIX = H * W_IMG            # 1296
J = 11                      # pixels per partition per frame
PPAD = 128 * J              # 1408 padded pixels per frame
NB = 512                    # histogram bins
LWIN = 101                  # lookup window
OUT_D = 128                 # fc output dim
FG = 8                      # frames per processing group
N_CORES = 8
SLABW = 228                 # sims band slab width: 128 + 101 - 1
PAD_VAL = 1 << 11           # pad pixel channel value; (>>5) = 64 -> no r match


def build_nc(BL: int, T: int):
    """Build the per-core program. BL = local batches, T = frames/batch."""
    nc = bacc.Bacc("TRN2")
    NF = BL * T                      # local frames
    assert NF % FG == 0
    NGROUP = NF // FG
    NTT = T // 128                   # t-tiles per batch

    fr = nc.dram_tensor("frames_pp", [NF, 128, 3 * J], dt.int32, kind="ExternalInput")
    iota_r = nc.dram_tensor("iota_r", [128, 8 * J], dt.bfloat16, kind="ExternalInput")
    iota_gb = nc.dram_tensor("iota_gb", [128, 64 * J], dt.bfloat16, kind="ExternalInput")
    wt = nc.dram_tensor("wt", [LWIN, OUT_D], dt.float32, kind="ExternalInput")
    bias_rep = nc.dram_tensor("bias_rep", [128, OUT_D], dt.float32, kind="ExternalInput")
    ident = nc.dram_tensor("ident", [128, 128], dt.float32, kind="ExternalInput")

    hist_d = nc.dram_tensor("hist_d", [NF, NB], dt.float32, kind="Internal")
    band_d = nc.dram_tensor("band_d", [BL, NTT, 128, SLABW], dt.float32, kind="Internal")
    out = nc.dram_tensor("out", [BL, T, OUT_D], dt.float32, kind="ExternalOutput")

    with TileContext(nc) as tc:
        with (
            tc.tile_pool(name="const", bufs=1) as cst,
            tc.tile_pool(name="fwork", bufs=3) as fwork,
            tc.tile_pool(name="iwork", bufs=2) as iwork,
            tc.tile_pool(name="ohp", bufs=2) as ohp,
            tc.tile_pool(name="hps", bufs=2, space="PSUM") as hps,
            tc.tile_pool(name="pps", bufs=2, space="PSUM") as pps,
            tc.tile_pool(name="post", bufs=2) as post,
            tc.tile_pool(name="xtp", bufs=1) as xtp,
        ):
            # ---- constants ----
            t_ir = cst.tile([128, 8 * J], dt.bfloat16)
            nc.sync.dma_start(out=t_ir[:], in_=iota_r[:])
            t_igb = cst.tile([128, 64 * J], dt.bfloat16)
            nc.sync.dma_start(out=t_igb[:], in_=iota_gb[:])
            t_wt = cst.tile([128, OUT_D], dt.float32)
            nc.sync.dma_start(out=t_wt[:LWIN, :], in_=wt[:])
            t_bias = cst.tile([128, OUT_D], dt.float32)
            nc.sync.dma_start(out=t_bias[:], in_=bias_rep[:])
            t_id = cst.tile([128, 128], dt.float32)
            nc.sync.dma_start(out=t_id[:], in_=ident[:])

            # ---- phase 1: histograms ----
            for g in range(NGROUP):
                f8 = fwork.tile([128, FG * 3 * J], dt.int32, tag="f8")
                nc.sync.dma_start(
                    out=f8.rearrange("p (f w) -> p f w", f=FG),
                    in_=fr[g * FG : (g + 1) * FG].transpose([1, 0, 2]),
                )
                f8v = f8.rearrange("p (f j c) -> p f j c", f=FG, j=J, c=C)
                ri = iwork.tile([128, FG * J], dt.int32, tag="ri")
                gbi = iwork.tile([128, FG * J], dt.int32, tag="gbi")
                tmp = iwork.tile([128, FG * J], dt.int32, tag="tmp")
                riv = ri.rearrange("p (f j) -> p f j", f=FG)
                gbiv = gbi.rearrange("p (f j) -> p f j", f=FG)
                tmpv = tmp.rearrange("p (f j) -> p f j", f=FG)
                nc.vector.tensor_scalar(
                    out=riv, in0=f8v[:, :, :, 0], scalar1=5, scalar2=None,
                    op0=Alu.logical_shift_right,
                )
                nc.vector.tensor_scalar(
                    out=gbiv, in0=f8v[:, :, :, 1], scalar1=5, scalar2=3,
                    op0=Alu.logical_shift_right, op1=Alu.logical_shift_left,
                )
                nc.vector.tensor_scalar(
                    out=tmpv, in0=f8v[:, :, :, 2], scalar1=5, scalar2=None,
                    op0=Alu.logical_shift_right,
                )
                nc.vector.tensor_add(gbi[:], gbi[:], tmp[:])
                rbf = iwork.tile([128, FG * J], dt.bfloat16, tag="rbf")
                nc.vector.tensor_copy(out=rbf[:], in_=ri[:])
                gbbf = iwork.tile([128, FG * J], dt.bfloat16, tag="gbbf")
                nc.vector.tensor_copy(out=gbbf[:], in_=gbi[:])

                ohr = ohp.tile([128, FG * 8 * J], dt.bfloat16, tag="ohr")
                ohgb = ohp.tile([128, FG * 64 * J], dt.bfloat16, tag="ohgb")
                nc.vector.tensor_tensor(
                    out=ohr.rearrange("p (f k j) -> p f k j", f=FG, k=8),
                    in0=rbf.rearrange("p (f j) -> p f j", f=FG)
                        .unsqueeze(2).broadcast_to([128, FG, 8, J]),
                    in1=t_ir.rearrange("p (k j) -> p k j", k=8)
                        .unsqueeze(1).broadcast_to([128, FG, 8, J]),
                    op=Alu.is_equal,
                )
                nc.vector.tensor_tensor(
                    out=ohgb.rearrange("p (f k j) -> p f k j", f=FG, k=64),
                    in0=gbbf.rearrange("p (f j) -> p f j", f=FG)
                        .unsqueeze(2).broadcast_to([128, FG, 64, J]),
                    in1=t_igb.rearrange("p (k j) -> p k j", k=64)
                        .unsqueeze(1).broadcast_to([128, FG, 64, J]),
                    op=Alu.is_equal,
                )

                ps = hps.tile([8, FG * 64], dt.float32, tag="hist")
                ohrv = ohr.rearrange("p (f k j) -> p f k j", f=FG, k=8)
                ohgbv = ohgb.rearrange("p (f k j) -> p f k j", f=FG, k=64)
                for f in range(FG):
                    for j in range(J):
                        nc.tensor.matmul(
                            ps[:, f * 64 : (f + 1) * 64],
                            ohrv[:, f, :, j],
                            ohgbv[:, f, :, j],
                            start=(j == 0),
                            stop=(j == J - 1),
                        )
                hs = fwork.tile([8, FG * 64], dt.float32, tag="hs")
                nc.scalar.copy(out=hs[:], in_=ps[:])
                nc.sync.dma_start(
                    out=hist_d[g * FG : (g + 1) * FG].rearrange(
                        "f (r gb) -> r f gb", r=8
                    ),
                    in_=hs.rearrange("r (f gb) -> r f gb", f=FG),
                )

            # ---- phase 2: normalize, sims band, fc ----
            for b in range(BL):
                xT = xtp.tile([128, 4 * T], dt.float32, tag="xT")
                for tt in range(NTT):
                    xt = post.tile([128, NB], dt.float32, tag="xt")
                    nc.sync.dma_start(
                        out=xt[:],
                        in_=hist_d[b * T + tt * 128 : b * T + (tt + 1) * 128],
                    )
                    sq = post.tile([128, NB], dt.float32, tag="sq")
                    ss = post.tile([128, 1], dt.float32, tag="ss")
                    nc.vector.tensor_mul(sq[:], xt[:], xt[:])
                    nc.vector.tensor_reduce(
                        out=ss[:], in_=sq[:], axis=mybir.AxisListType.X, op=Alu.add
                    )
                    nrm = post.tile([128, 1], dt.float32, tag="nrm")
                    nc.scalar.activation(out=nrm[:], in_=ss[:], func=Act.Sqrt)
                    rn = post.tile([128, 1], dt.float32, tag="rn")
                    nc.vector.reciprocal(rn[:], nrm[:])
                    nc.vector.tensor_scalar_mul(xt[:], xt[:], rn[:])
                    for kk in range(4):
                        pt = pps.tile([128, 128], dt.float32, tag="pp")
                        nc.tensor.transpose(
                            pt[:], xt[:, kk * 128 : (kk + 1) * 128], t_id[:]
                        )
                        nc.scalar.copy(
                            out=xT[:, kk * T + tt * 128 : kk * T + (tt + 1) * 128],
                            in_=pt[:],
                        )

                for tt in range(NTT):
                    t0 = tt * 128
                    j0 = max(0, 50 - t0)
                    j1 = min(SLABW, T - t0 + 50)
                    s0 = t0 - 50 + j0
                    pss = pps.tile([128, SLABW], dt.float32, tag="sims")
                    for kk in range(4):
                        nc.tensor.matmul(
                            pss[:, j0:j1],
                            xT[:, kk * T + t0 : kk * T + t0 + 128],
                            xT[:, kk * T + s0 : kk * T + s0 + (j1 - j0)],
                            start=(kk == 0),
                            stop=(kk == 3),
                        )
                    slab = post.tile([128, SLABW], dt.float32, tag="slab")
                    if j0 > 0:
                        nc.vector.memset(slab[:, :j0], 0.0)
                    if j1 < SLABW:
                        nc.vector.memset(slab[:, j1:], 0.0)
                    nc.scalar.copy(out=slab[:, j0:j1], in_=pss[:, j0:j1])
                    nc.sync.dma_start(out=band_d[b, tt], in_=slab[:])

                for tt in range(NTT):
                    banded = post.tile([128, LWIN], dt.float32, tag="bnd")
                    base = band_d[b, tt]          # [128, SLABW]
                    diag = bass.AP(
                        base.tensor,
                        base.offset,
                        [[SLABW + 1, 128], [1, LWIN]],
                    )
                    nc.sync.dma_start(out=banded[:], in_=diag)
                    pbt = pps.tile([128, 128], dt.float32, tag="pp")
                    nc.tensor.transpose(pbt[:LWIN, :], banded[:], t_id[:])
                    bT = post.tile([128, 128], dt.float32, tag="bT")
                    nc.scalar.copy(out=bT[:LWIN, :], in_=pbt[:LWIN, :])
                    po = pps.tile([128, OUT_D], dt.float32, tag="pp")
                    nc.tensor.matmul(
                        po[:], bT[:LWIN, :], t_wt[:LWIN, :], start=True, stop=True
                    )
                    ob = post.tile([128, OUT_D], dt.float32, tag="ob")
                    nc.vector.tensor_add(ob[:], po[:], t_bias[:])
                    nc.vector.tensor_scalar_max(ob[:], ob[:], 0.0)
                    nc.sync.dma_start(
                        out=out[b, tt * 128 : (tt + 1) * 128, :], in_=ob[:]
                    )

    nc.compile()
    return nc


def prep_frames_core(frames_core: np.ndarray) -> np.ndarray:
    """[BL, T, H, W, C] int32 -> [BL*T, 128, 3*J] padded layout."""
    BL, T = frames_core.shape[:2]
    f = frames_core.reshape(BL * T, NPIX, C)
    pad = np.full((BL * T, PPAD - NPIX, C), PAD_VAL, dtype=np.int32)
    fp = np.concatenate([f, pad], axis=1)          # [NF, 1408, 3]
    return np.ascontiguousarray(fp.reshape(BL * T, 128, J * C))


def make_consts():
    import ml_dtypes
    ir = np.broadcast_to(np.repeat(np.arange(8), J), (128, 8 * J))
    igb = np.broadcast_to(np.repeat(np.arange(64), J), (128, 64 * J))
    return {
        "iota_r": np.ascontiguousarray(ir).astype(ml_dtypes.bfloat16),
        "iota_gb": np.ascontiguousarray(igb).astype(ml_dtypes.bfloat16),
        "ident": np.eye(128, dtype=np.float32),
    }


_CACHE = {}


def _get_nc(BL, T):
    key = (BL, T)
    if key not in _CACHE:
        _CACHE[key] = build_nc(BL, T)
    return _CACHE[key]


def kernel(frames: np.ndarray, W: np.ndarray, b: np.ndarray, _profile=False):
    frames = np.asarray(frames, dtype=np.int32)
    W = np.asarray(W, dtype=np.float32)
    b = np.asarray(b, dtype=np.float32)
    B, T = frames.shape[:2]
    BL = B // N_CORES
    nc = _get_nc(BL, T)

    consts = make_consts()
    wt = np.ascontiguousarray(W.T)                          # [101, 128]
    bias_rep = np.ascontiguousarray(
        np.broadcast_to(b[None, :], (128, OUT_D))
    ).astype(np.float32)

    in_maps = []
    for c in range(N_CORES):
        fc = frames[c * BL : (c + 1) * BL]
        in_maps.append(
            {
                "frames_pp": prep_frames_core(fc),
                "wt": wt,
                "bias_rep": bias_rep,
                **consts,
            }
        )

    res = bass_utils.run_bass_kernel_spmd(
        nc, in_maps, core_ids=list(range(N_CORES)), trace=_profile
    )
    outs = [res.results[c]["out"] for c in range(N_CORES)]
    full = np.concatenate(outs, axis=0)                     # [B, T, OUT_D]
    if _profile:
        return full, res
    return full
